# revision 21
# baseline (speedup 1.0000x reference)
"""Causal single-head attention on 8 TRN2 NeuronCores, data-parallel over batch.

Per core (one batch element): x [T=2048, C=1024], weights [C, H=128].
  q = x@Wq + bq ; k = x@Wk + bk ; v = x@Wv + bv
  out = softmax(mask(q k^T / sqrt(H))) @ v

Layout strategy (no on-device transposes anywhere):
  - host passes x^T [C, T] bf16; projections contract C on partitions:
      qT, kT [H, T] (stationary = W[c,h]), v [T, H] (stationary = xT[c,t128])
  - scores computed transposed, S'[s, t] = k q^T, via stationary kT[:, s128]
  - softmax sums via a ones-column appended to v: the PV matmul per t-chunk
    yields both sum_s P'[s,t] v[s,h] and sum_s P'[s,t]
  - causal: blocks above the diagonal are skipped, diagonal s-tiles compute
    only the valid t' range, one [128,128] triangular mask on the mixed chunk
  - matmul inputs bf16 (fp32 PSUM accumulation); output + biases fp32

Scheduling (v2):
  - projection PSUM->SBUF copies on VectorE (ScalarE stays exp-only; ACT is
    the per-chunk critical path in the last attention block)
  - filler rebalance: q-proj(j+1) interleaves into attention(j); k/v-proj(j)
    interleave into attention(j)'s own pre-diagonal iterations, so the last
    (largest) attention block still has PE filler while ACT churns exps
  - x chunk DMAs split across the sync+gpsimd queues (2x bandwidth), outputs
    also on both; masked diagonal block written to a separate tile so only
    the mixed PV matmul waits on the mask
"""

import numpy as np
import ml_dtypes

import concourse.bass as bass
import concourse.mybir as mybir
import concourse.tile as tile
from concourse.bass_utils import run_bass_kernel_spmd

F32 = mybir.dt.float32
BF16 = mybir.dt.bfloat16
AF = mybir.ActivationFunctionType

B, T, C, H = 8, 2048, 1024, 128
P = 128
CT = C // P        # 8 contraction tiles
TBLK = 512         # t-block / projection chunk width
NBLK = T // TBLK   # 4
NST = T // P       # 16 s-tiles
SCALE = 1.0 / float(np.sqrt(H))

N_CORES = 8


def _split_multiwaits(nc, max_waits=1):
    """walrus in this image rejects >1 sem wait on one instruction; hoist
    extras onto single-wait NOPs placed just before on the same engine."""
    n_new = 0
    for fn in nc.m.functions:
        for bb in fn.blocks:
            new_insts = []
            for ins in bb.instructions:
                si = ins.sync_info
                if si is not None and si.on_wait and len(si.on_wait) > max_waits:
                    waits = list(si.on_wait)
                    for w in waits[:-max_waits]:
                        n_new += 1
                        new_insts.append(
                            mybir.InstNoOp(
                                name=f"I-waitsplit-{n_new}",
                                engine=ins.engine,
                                ins=[],
                                outs=[],
                                sync_info=mybir.SyncInfo(on_wait=[w], on_update=[]),
                            )
                        )
                    ins.sync_info = mybir.SyncInfo(
                        on_wait=waits[-max_waits:],
                        on_update=list(si.on_update or []),
                    )
                new_insts.append(ins)
            bb.instructions = new_insts
    return n_new


def _trim_tail(nc):
    """Drop the second all-engine barrier after the tail semaphore
    range-clear: nothing executes after it inside the NEFF, the clear is
    already ordered after every engine's last use by the first barrier,
    so the post-clear round only adds ~1us of teardown."""
    for fn in nc.m.functions:
        for bb in fn.blocks:
            last_isa = None
            for i, ins in enumerate(bb.instructions):
                if type(ins).__name__ == "InstISA":
                    last_isa = i
            if last_isa is not None:
                bb.instructions = bb.instructions[: last_isa + 1]


def _build(split=True, with_bias=False):
    nc = bass.Bass()
    xT = nc.declare_dram_parameter("xT", [C, T], BF16, isOutput=False)
    wqkv = nc.declare_dram_parameter("wqkv", [C, 3 * H], BF16, isOutput=False)
    if with_bias:
        bqk = nc.declare_dram_parameter("bqk", [H, 2], F32, isOutput=False)
        bv = nc.declare_dram_parameter("bv", [H], F32, isOutput=False)
    out = nc.declare_dram_parameter("out", [T, H], F32, isOutput=True)

    with (
        tile.TileContext(nc) as tc,
        tc.tile_pool(name="singles", bufs=1) as singles,
        tc.tile_pool(name="xbfp", bufs=3) as xbfp,
        tc.tile_pool(name="psbp", bufs=4) as psbp,
        tc.tile_pool(name="pdbp", bufs=2) as pdbp,
        tc.tile_pool(name="osbp", bufs=4) as osbp,
        tc.tile_pool(name="rsbp", bufs=4) as rsbp,
        tc.tile_pool(name="ps_prj", bufs=2, space="PSUM") as ps_prj,
        tc.tile_pool(name="ps_s", bufs=2, space="PSUM") as ps_s,
        tc.tile_pool(name="ps_o", bufs=1, space="PSUM") as ps_o,
    ):
        # ---- PE warmup: release the HAM clock gate while DMAs land ----
        # memset on the otherwise-idle vector engine so warm matmuls can
        # start as early as possible
        warm = singles.tile([P, TBLK], BF16)
        nc.vector.memset(warm[:], 0.0)

        def warm_mms(n, w=P):
            ps_warm = ps_s.tile([P, TBLK], F32, tag="ps", name="ps_warm")
            for _ in range(n):
                nc.tensor.matmul(
                    ps_warm[:, 0:w], warm[:, 0:P], warm[:, 0:w],
                    start=True, stop=True,
                )

        # ---- constants ----
        w_bf = singles.tile([P, CT, 3 * H], BF16)

        if with_bias:
            bqk_sb = singles.tile([P, 2], F32)
            nc.gpsimd.dma_start(bqk_sb[:], bqk[:, :])
            bv_rep = singles.tile([P, H], F32)
            bv_ap = bv[:]
            nc.gpsimd.dma_start(
                bv_rep[:],
                bass.AP(
                    tensor=bv_ap.tensor, offset=bv_ap.offset, ap=[[0, P], [1, H]]
                ),
            )

        qT_sb = singles.tile([P, T], BF16)   # [h, t]
        kT_sb = singles.tile([P, T], BF16)   # [h, t]
        v_sb = singles.tile([P, NST, 132], BF16)  # [s128, s-tile, h | ones]
        mask = singles.tile([P, P], BF16)

        # ~11 cold N=512 matmuls bridge the PE from program start to the
        # first x tiles landing (~4.5us) with no idle gap, so the HAM
        # activity window fires as early as possible
        warm_mms(11, w=TBLK)

        def dma_chunk(j, wave=False, with_w=False):
            """Per-c-tile x DMAs. The prologue wave (chunks 0+1 and w) is
            striped over THREE queues (sync+gpsimd+scalar) to saturate HBM;
            mid-kernel prefetches use sync+gpsimd only (scalar carries the
            latency-critical exp stream). w tiles interleave with x0 so
            q(0)'s o-th matmul deps arrive together in o order."""
            x_bf = xbfp.tile([P, CT, TBLK], BF16, tag="x_bf", name="x_bf")
            t0 = j * TBLK
            engs3 = [nc.sync, nc.gpsimd, nc.scalar]
            for o in range(CT):
                eng = engs3[o % 3] if wave else (
                    nc.sync if o % 2 == 0 else nc.gpsimd
                )
                if with_w:
                    eng.dma_start(w_bf[:, o, :], wqkv[o * P : (o + 1) * P, :])
                eng.dma_start(
                    x_bf[:, o, :], xT[o * P : (o + 1) * P, t0 : t0 + TBLK]
                )
            return x_bf

        def setup_constants():
            """gpsimd-queue work that must trail the x DMAs (keeps the queue
            free for the first chunks): triangular mask
            mask[i, t''] = 1.0 if t'' >= i else 0.0, ones columns for v
            (only the appended column; [0:H] is overwritten by v copies)."""
            nc.gpsimd.memset(mask[:], 1.0)
            nc.gpsimd.affine_select(
                out=mask[:],
                in_=mask[:],
                compare_op=mybir.AluOpType.is_ge,
                fill=0.0,
                base=0,
                pattern=[[1, P]],
                channel_multiplier=-1,
            )
            nc.gpsimd.memset(v_sb[:, :, H:132], 1.0)

        def copy_qk(dst, src, col):
            """PSUM->SBUF projection copy. VectorE when no bias (keeps ACT
            exp-only); ScalarE with fused per-partition bias otherwise."""
            if with_bias:
                nc.scalar.activation(
                    dst, src, AF.Identity, bias=bqk_sb[:, col : col + 1]
                )
            else:
                nc.vector.tensor_copy(dst, src)

        def gen_q(j, x_bf):
            """q-projection for t-chunk j, yielded in PE-unit steps."""
            t0 = j * TBLK
            pqk = ps_prj.tile([P, TBLK], F32, tag="prj", name="pq")
            for o in range(CT):
                nc.tensor.matmul(
                    pqk[:], w_bf[:, o, 0:H], x_bf[:, o, :],
                    start=(o == 0), stop=(o == CT - 1),
                )
                yield
            copy_qk(qT_sb[:, t0 : t0 + TBLK], pqk[:], 0)
            yield

        def gen_k(j, x_bf):
            t0 = j * TBLK
            pqk = ps_prj.tile([P, TBLK], F32, tag="prj", name="pk")
            for o in range(CT):
                nc.tensor.matmul(
                    pqk[:], w_bf[:, o, H : 2 * H], x_bf[:, o, :],
                    start=(o == 0), stop=(o == CT - 1),
                )
                yield
            copy_qk(kT_sb[:, t0 : t0 + TBLK], pqk[:], 1)
            yield

        def gen_v(j, x_bf):
            pvv = ps_prj.tile([P, 4, H], F32, tag="prj", name="pv")
            for m4 in range(4):
                for o in range(CT):
                    nc.tensor.matmul(
                        pvv[:, m4, :],
                        x_bf[:, o, m4 * P : (m4 + 1) * P],
                        w_bf[:, o, 2 * H : 3 * H],
                        start=(o == 0), stop=(o == CT - 1),
                    )
                    if o % 2 == 1:
                        yield
                # per-tile copy so PV consumers unblock as tiles complete
                nc.vector.tensor_copy(
                    v_sb[:, 4 * j + m4, 0:H], pvv[:, m4, :]
                )
            yield

        def drain(gen):
            if gen is not None:
                for _ in gen:
                    pass

        # ---- prologue: q(0)+k(0); v(0) streams into att(0) as filler ----
        x_cur = dma_chunk(0, wave=True, with_w=True)
        # dummy exp between the wave DMAs on the scalar queue: pulls the
        # ~2.7us ACT exp-table load off the first softmax's critical path
        act_warm = singles.tile([P, 1], F32)
        nc.scalar.activation(act_warm[:], warm[:, 0:1], AF.Exp)
        x_nxt = dma_chunk(1, wave=True)
        setup_constants()
        drain(gen_q(0, x_cur))
        drain(gen_k(0, x_cur))

        for j in range(NBLK):
            t0 = j * TBLK
            n_s = 4 * (j + 1)

            # filler generators for this attention block:
            #   kv: k(j)/v(j) consumed in att(j)'s pre-diagonal iterations
            #       (k needed by S'(4j) at iter 4j; v tiles stream to PV)
            #   qn: q(j+1) spread across all iterations (needed by att(j+1))
            if j == 0:
                kv_units, kv_left, kv_dl = gen_v(0, x_cur), 17, 3
            else:
                def kv_gen(xb=x_cur, jj=j):
                    yield from gen_k(jj, xb)
                    yield from gen_v(jj, xb)
                kv_units, kv_left, kv_dl = kv_gen(), 26, 4 * j
            if j + 1 < NBLK:
                x_new = dma_chunk(j + 2) if j + 2 < NBLK else None
                qn_units, qn_left = gen_q(j + 1, x_nxt), 9
            else:
                x_new, qn_units, qn_left = None, None, 0

            po_tiles = [
                ps_o.tile([P, 132], F32, tag=f"po{c}", name=f"po{c}")
                for c in range(4)
            ]

            def pv_mms(m, p_sb, pd):
                r = m - 4 * j
                # masked (c == r) matmul last: the others depend only on exp,
                # not on the DVE mask multiply
                order = list(range(max(r, 0) + 1, 4)) + ([r] if r >= 0 else [0])
                if r < 0:
                    order = list(range(0, 4))
                for c in order:
                    stat = (
                        pd[:, 0:P] if (pd is not None and c == r)
                        else p_sb[:, c * P : (c + 1) * P]
                    )
                    nc.tensor.matmul(
                        po_tiles[c][:, 0 : H + 1],
                        stat,
                        v_sb[:, m, 0 : H + 1],
                        start=(m == 0), stop=(m == 4 * j + c),
                    )

            def epilogue(c):
                po = po_tiles[c]
                rec = rsbp.tile([P, 1], F32, tag="rec", name="rec")
                nc.vector.reciprocal(rec[:], po[:, H : H + 1])
                o_sb = osbp.tile([P, H], F32, tag="o_sb", name="o_sb")
                nc.vector.tensor_scalar_mul(o_sb[:], po[:, 0:H], rec[:])
                if with_bias:
                    nc.vector.tensor_add(o_sb[:], o_sb[:], bv_rep[:])
                base = t0 + c * P
                # last chunk: spread the 4 epilogue DMAs over queues, keeping
                # the final one OFF sync so sync's teardown wait-chain starts
                # while the last transfers are still in flight
                eng = (
                    [nc.sync, nc.gpsimd, nc.scalar, nc.gpsimd][c]
                    if j == NBLK - 1 else nc.sync
                )
                eng.dma_start(out[base : base + P, :], o_sb[:])

            def post_pv(m, p_sb, pd):
                pv_mms(m, p_sb, pd)
                c_done = m - 4 * j
                if c_done >= 0:
                    epilogue(c_done)

            prev = None
            for m in range(n_s):
                r = m - 4 * j
                lo = P * max(r, 0)
                ps = ps_s.tile([P, TBLK], F32, tag="ps", name="ps")
                nc.tensor.matmul(
                    ps[:, lo:TBLK],
                    kT_sb[:, m * P : (m + 1) * P],
                    qT_sb[:, t0 + lo : t0 + TBLK],
                    start=True, stop=True,
                )
                p_sb = psbp.tile([P, TBLK], BF16, tag="p_sb", name="p_sb")
                nc.scalar.activation(
                    p_sb[:, lo:TBLK], ps[:, lo:TBLK], AF.Exp, scale=SCALE
                )
                pd = None
                if r >= 0:
                    # mask the mixed diagonal block into its own tile so only
                    # the c==r PV matmul depends on the mask multiply
                    pd = pdbp.tile([P, P], BF16, tag="pd", name="pd")
                    nc.vector.tensor_mul(
                        pd[:], p_sb[:, lo : lo + P], mask[:]
                    )
                if prev is not None:
                    post_pv(*prev)
                prev = (m, p_sb, pd)

                # interleave filler units: k/v(j) into pre-diagonal iters,
                # q(j+1) across the whole block
                if kv_left:
                    k_units = -(-kv_left // max(kv_dl - m, 1))
                    for _ in range(k_units):
                        try:
                            next(kv_units)
                            kv_left -= 1
                        except StopIteration:
                            kv_left = 0
                            break
                if qn_left:
                    k_units = -(-qn_left // (n_s - m))
                    for _ in range(k_units):
                        try:
                            next(qn_units)
                            qn_left -= 1
                        except StopIteration:
                            qn_left = 0
                            break
            post_pv(*prev)
            drain(kv_units)
            drain(qn_units)
            x_cur, x_nxt = x_nxt, x_new

    if split:
        _trim_tail(nc)
        _split_multiwaits(nc)
    return nc


_NC_CACHE = {}


def _get_nc(with_bias=False):
    key = bool(with_bias)
    if key not in _NC_CACHE:
        _NC_CACHE[key] = _build(with_bias=key)
    return _NC_CACHE[key]


def _prepare_in_maps(batch_x, Wq, bq, Wk, bk, Wv, bv, with_bias):
    wqkv = np.ascontiguousarray(
        np.concatenate([np.asarray(Wq), np.asarray(Wk), np.asarray(Wv)], axis=1)
    ).astype(ml_dtypes.bfloat16)
    extra = {}
    if with_bias:
        extra["bqk"] = np.ascontiguousarray(
            np.stack([np.asarray(bq), np.asarray(bk)], axis=1).astype(np.float32)
        )
        extra["bv"] = np.ascontiguousarray(np.asarray(bv).astype(np.float32))
    bx = np.asarray(batch_x)
    return [
        {
            "xT": np.ascontiguousarray(bx[i].T).astype(ml_dtypes.bfloat16),
            "wqkv": wqkv,
            **extra,
        }
        for i in range(N_CORES)
    ]


def _needs_bias(bq, bk, bv):
    return bool(
        np.any(np.asarray(bq)) or np.any(np.asarray(bk)) or np.any(np.asarray(bv))
    )


def kernel(batch_x, Wq, bq, Wk, bk, Wv, bv):
    wb = _needs_bias(bq, bk, bv)
    nc = _get_nc(with_bias=wb)
    in_maps = _prepare_in_maps(batch_x, Wq, bq, Wk, bk, Wv, bv, with_bias=wb)
    res = run_bass_kernel_spmd(nc, in_maps, core_ids=list(range(N_CORES)))
    return np.stack([res.results[i]["out"] for i in range(N_CORES)], axis=0)
